# revision 4
# baseline (speedup 1.0000x reference)
"""GRU classifier Bass kernel v3 (per-core program, SPMD over 8 cores).

Layout: gate units on partitions, batch on free dim. Per-core batch B=32.
Gate slices m=0..5 cover 128 units each: m 0,1 -> r ; m 2,3 -> z (all z-side
weights/biases NEGATED at pack time so sigmoid gives z1=1-z directly);
m 4,5 -> n.

Per step, three per-gate PSUM banks (each [128, 2*B]):
  Pr/Pz init: ident-MM carrying xg (+bias) ; Pn init: ident-MM carrying b_hh_n
  then 4 W-MMs per gate (2 slices x 2 kc chunks), order r, n, z (r first so
  its sigmoid starts after 4 MMs; n second so prod is gated as early as
  possible; z last since its sigmoid is consumed at the tail).
Chain (ACT program order sig_r -> tanh -> sig_z1: z1's psum stops last, so
queueing it before tanh would head-of-line-block the ACT FIFO):
  r = sigmoid(Pr)
  prod = Pn * r ; nin = prod + xg_n ; n = tanh(nin)
  d  = n*(1/S) - h_s             (scalar_tensor_tensor, runs during z1)
  z1 = sigmoid(Pz)               (z psum pre-negated)
  zd = z1 * d ; h'_s = zd + h_s  (h' = z1*(n/S - h_s) + h_s)
Hidden state kept scaled: h_s = h / S (S=64) so W_hh can be stored as
fp8e4m3 * S for full mantissa use; FC weights pre-scaled by S.
"""
import numpy as np
import ml_dtypes
import concourse.bass as bass
import concourse.bacc as bacc
import concourse.mybir as mybir
import concourse.tile as tile

BF16 = mybir.dt.bfloat16
F32 = mybir.dt.float32
FP8 = mybir.dt.float8e4
AF = mybir.ActivationFunctionType
ALU = mybir.AluOpType

USE_FP8 = True
HSCALE = 64.0

B = 32          # batch per core
H = 256
G = 768
I_IN = 512
N_C = 101
NSLICE = 6      # gate slices of 128
KC_H = 2        # hidden contraction chunks
KC_I = 4        # input contraction chunks

W_DT = FP8 if USE_FP8 else BF16
W_NP = ml_dtypes.float8_e4m3 if USE_FP8 else ml_dtypes.bfloat16


def build_nc(T=512, TBLK=64, n_cores=8, repeat=1):
    NBLK = T // TBLK
    assert T % TBLK == 0
    nc = bacc.Bacc("TRN2", target_bir_lowering=False, debug=False,
                   num_devices=n_cores)

    xT = nc.dram_tensor("xT", [I_IN, T, B], BF16, kind="ExternalInput").ap()
    wih = nc.dram_tensor("wih", [128, KC_I, NSLICE, 128], BF16, kind="ExternalInput").ap()
    whh = nc.dram_tensor("whh", [128, KC_H, NSLICE, 128], W_DT, kind="ExternalInput").ap()
    ident_d = nc.dram_tensor("ident", [128, 128], BF16, kind="ExternalInput").ap()
    biasn_d = nc.dram_tensor("biasn", [128, 2 * B], BF16, kind="ExternalInput").ap()
    brz_d = nc.dram_tensor("brz", [128, 4], F32, kind="ExternalInput").ap()
    bihn_d = nc.dram_tensor("bihn", [128, 2], F32, kind="ExternalInput").ap()
    fcw_d = nc.dram_tensor("fcw", [128, KC_H, N_C], BF16, kind="ExternalInput").ap()
    fcb_d = nc.dram_tensor("fcb", [N_C, 1], F32, kind="ExternalInput").ap()
    out_d = nc.dram_tensor("out", [N_C, B], F32, kind="ExternalOutput").ap()

    with tile.TileContext(nc) as tc:
        for _rep in range(repeat):
            _body(tc, T, TBLK, NBLK, xT, wih, whh, ident_d, biasn_d, brz_d,
                  bihn_d, fcw_d, fcb_d, out_d)
    nc.compile()
    return nc


def _body(tc, T, TBLK, NBLK, xT, wih, whh, ident_d, biasn_d, brz_d, bihn_d,
          fcw_d, fcb_d, out_d):
    nc = tc.nc
    from contextlib import ExitStack
    ctx = ExitStack()
    const = ctx.enter_context(tc.tile_pool(name="const", bufs=1))
    xtp = ctx.enter_context(tc.tile_pool(name="xt", bufs=2))
    xgp = ctx.enter_context(tc.tile_pool(name="xg", bufs=2))
    ew = ctx.enter_context(tc.tile_pool(name="ew", bufs=3))
    hp = ctx.enter_context(tc.tile_pool(name="h", bufs=2))
    psr = ctx.enter_context(tc.tile_pool(name="psr", bufs=2, space="PSUM"))
    psz = ctx.enter_context(tc.tile_pool(name="psz", bufs=2, space="PSUM"))
    psn = ctx.enter_context(tc.tile_pool(name="psn", bufs=2, space="PSUM"))
    psp = ctx.enter_context(tc.tile_pool(name="psp", bufs=2, space="PSUM"))

    # ---- constants to SBUF ----
    wih_t = const.tile([128, KC_I, NSLICE, 128], BF16)
    nc.sync.dma_start(out=wih_t[:], in_=wih[:])
    whh_t = const.tile([128, KC_H, NSLICE, 128], W_DT)
    nc.sync.dma_start(out=whh_t[:], in_=whh[:])
    ident = const.tile([128, 128], BF16)
    nc.sync.dma_start(out=ident[:], in_=ident_d[:])
    biasn = const.tile([128, 2 * B], BF16)
    nc.sync.dma_start(out=biasn[:], in_=biasn_d[:])
    brz = const.tile([128, 4], F32)
    nc.sync.dma_start(out=brz[:], in_=brz_d[:])
    bihn = const.tile([128, 2], F32)
    nc.sync.dma_start(out=bihn[:], in_=bihn_d[:])
    fcw = const.tile([128, KC_H, N_C], BF16)
    nc.sync.dma_start(out=fcw[:], in_=fcw_d[:])
    fcb = const.tile([N_C, 1], F32)
    nc.sync.dma_start(out=fcb[:], in_=fcb_d[:])

    # ---- hidden state (ping-pong), scaled h_s = h / HSCALE ----
    h_tiles = [hp.tile([128, KC_H * B], BF16, tag="h", name=f"h{i}")
               for i in range(2)]
    nc.vector.memset(h_tiles[0][:], 0.0)

    # ---- projection: one block of TBLK steps into an xg sbuf tile ----
    NS_T = 16                       # timesteps per psum (N = NS_T*B = 512)
    NSUB = TBLK // NS_T

    def proj_block_ops(blk):
        t0 = blk * TBLK
        xt_t = xtp.tile([128, KC_I, TBLK, B], BF16, tag="xt")
        xg_t = xgp.tile([128, TBLK, NSLICE, B], BF16, tag="xg")

        def dma_one(ic):
            nc.sync.dma_start(out=xt_t[:, ic], in_=xT[ic * 128:(ic + 1) * 128,
                                                      t0:t0 + TBLK, :])
        for ic in range(KC_I):
            yield lambda ic=ic: dma_one(ic)

        # Yield ONE matmul (or evac) per item so the per-step interleave
        # never inserts a >215ns contiguous PE chunk into the recurrence.
        for m in range(NSLICE):
            for ns in range(NSUB):
                holder = {}

                def mm_one(m=m, ns=ns, ic=0, holder=holder):
                    if ic == 0:
                        holder["ps"] = psp.tile([128, NS_T, B], F32,
                                                tag="psp",
                                                name=f"pp{m}_{ns}")
                    nc.tensor.matmul(
                        holder["ps"][:], lhsT=wih_t[:, ic, m, :],
                        rhs=xt_t[:, ic, ns * NS_T:(ns + 1) * NS_T, :],
                        start=(ic == 0), stop=(ic == KC_I - 1))
                for ic in range(KC_I):
                    yield (lambda m=m, ns=ns, ic=ic, holder=holder:
                           mm_one(m, ns, ic, holder))

                def evac(m=m, ns=ns, holder=holder):
                    ps = holder["ps"]
                    dst = xg_t[:, ns * NS_T:(ns + 1) * NS_T, m, :]
                    if m >= 4:
                        nc.scalar.activation(dst, ps[:], AF.Identity,
                                             bias=bihn[:, m - 4:m - 3])
                    else:
                        nc.vector.tensor_scalar_add(out=dst, in0=ps[:],
                                                    scalar1=brz[:, m:m + 1])
                yield evac
        yield ("done", xg_t)

    # ---- recurrence step ----
    def gru_step(t, xg_t, h_nxt):
        tl = t % TBLK
        h_cur = h_tiles[t % 2]
        Pr = psr.tile([128, 2 * B], F32, tag="pr", name=f"pr{t}")
        Pn = psn.tile([128, 2 * B], F32, tag="pn", name=f"pn{t}")
        Pz = psz.tile([128, 2 * B], F32, tag="pz", name=f"pz{t}")
        # init MMs (ident stationary, carries xg / bias into psum)
        nc.tensor.matmul(Pr[:], lhsT=ident[:], rhs=xg_t[:, tl, 0:2, :],
                         start=True, stop=False)
        nc.tensor.matmul(Pn[:], lhsT=ident[:], rhs=biasn[:],
                         start=True, stop=False)
        nc.tensor.matmul(Pz[:], lhsT=ident[:], rhs=xg_t[:, tl, 2:4, :],
                         start=True, stop=False)
        # W-MMs: r first (sigmoid asap), then n (prod gate), then z
        for gate_ms, P in (((0, 1), Pr), ((4, 5), Pn), ((2, 3), Pz)):
            for m in gate_ms:
                s = m & 1
                for kc in range(KC_H):
                    nc.tensor.matmul(
                        P[:, s * B:(s + 1) * B],
                        lhsT=whh_t[:, kc, m, :],
                        rhs=h_cur[:, kc * B:(kc + 1) * B],
                        start=False,
                        stop=(m == gate_ms[1] and kc == KC_H - 1))
        # chain. ACT program order: sig r -> tanh -> sig z1 (z1's input psum
        # stops last; queueing it before tanh head-of-line-blocks the ACT
        # FIFO for ~200ns/step). Tail uses h' = z1*(n/S - h_s) + h_s so only
        # two DVE ops follow z1.
        r = ew.tile([128, 2 * B], BF16, tag="sig", name=f"r{t}")
        nc.scalar.activation(r[:], Pr[:], AF.Sigmoid)
        prod = ew.tile([128, 2 * B], BF16, tag="prod")
        nc.vector.tensor_mul(out=prod[:], in0=Pn[:], in1=r[:])
        nin = ew.tile([128, 2 * B], BF16, tag="nin")
        nc.vector.tensor_add(out=nin[:], in0=prod[:], in1=xg_t[:, tl, 4:6, :])
        n_t = ew.tile([128, 2 * B], BF16, tag="n")
        nc.scalar.activation(n_t[:], nin[:], AF.Tanh)
        d_t = ew.tile([128, 2 * B], BF16, tag="d")
        nc.vector.scalar_tensor_tensor(out=d_t[:], in0=n_t[:],
                                       scalar=1.0 / HSCALE, in1=h_cur[:],
                                       op0=ALU.mult, op1=ALU.subtract)
        z1 = ew.tile([128, 2 * B], BF16, tag="z1")
        nc.scalar.activation(z1[:], Pz[:], AF.Sigmoid)
        zd = ew.tile([128, 2 * B], BF16, tag="zd")
        nc.vector.tensor_mul(out=zd[:], in0=z1[:], in1=d_t[:])
        nc.vector.tensor_add(out=h_nxt[:], in0=zd[:], in1=h_cur[:])

    # ---- main pipeline ----
    proj_gens = [proj_block_ops(b) for b in range(NBLK)]
    xg_tiles = [None] * NBLK
    for item in proj_gens[0]:
        if isinstance(item, tuple):
            xg_tiles[0] = item[1]
        else:
            item()

    for blk in range(NBLK):
        nxt = proj_gens[blk + 1] if blk + 1 < NBLK else None
        pending = list(nxt) if nxt is not None else []
        per_step = (len(pending) + TBLK - 1) // TBLK if pending else 0
        pi = 0
        for tl in range(TBLK):
            t = blk * TBLK + tl
            h_nxt = h_tiles[(t + 1) % 2]
            gru_step(t, xg_tiles[blk], h_nxt)
            for _ in range(per_step):
                if pi < len(pending):
                    item = pending[pi]; pi += 1
                    if isinstance(item, tuple):
                        xg_tiles[blk + 1] = item[1]
                    else:
                        item()
        while pi < len(pending):
            item = pending[pi]; pi += 1
            if isinstance(item, tuple):
                xg_tiles[blk + 1] = item[1]
            else:
                item()

    # ---- FC head: out[c, b] = fc_w @ h_T  (fcw pre-scaled by HSCALE) ----
    hT = h_tiles[T % 2]
    pfc = psp.tile([N_C, B], F32, tag="psp")
    for kc in range(KC_H):
        nc.tensor.matmul(pfc[:], lhsT=fcw[:, kc, :],
                         rhs=hT[:, kc * B:(kc + 1) * B],
                         start=(kc == 0), stop=(kc == KC_H - 1))
    ofc = ew.tile([N_C, B], F32, tag="ofc")
    nc.scalar.activation(ofc[:], pfc[:], AF.Identity, bias=fcb[:])
    nc.sync.dma_start(out=out_d[:], in_=ofc[:])
    ctx.close()


# ---------------- host-side packing ----------------

def pack_inputs(x_shard, W_ih, W_hh, b_ih, b_hh, fc_w, fc_b):
    """x_shard: [B, T, I] fp32. Returns dict of np arrays for one core."""
    bf = ml_dtypes.bfloat16
    Bs, T, Iin = x_shard.shape
    assert Bs == B and Iin == I_IN
    xT = np.ascontiguousarray(x_shard.transpose(2, 1, 0)).astype(bf)  # [I,T,B]

    # z-gate (units 256..511 of the 768) fully negated
    gsign = np.ones((G,), np.float32)
    gsign[256:512] = -1.0

    WihT = (W_ih * gsign[:, None]).T.astype(np.float32)    # [I, G]
    wih_t = np.zeros((128, KC_I, NSLICE, 128), np.float32)
    for ic in range(KC_I):
        for m in range(NSLICE):
            wih_t[:, ic, m, :] = WihT[ic * 128:(ic + 1) * 128,
                                      m * 128:(m + 1) * 128]
    WhhT = (W_hh * gsign[:, None]).T.astype(np.float32) * HSCALE  # [H, G]
    whh_t = np.zeros((128, KC_H, NSLICE, 128), np.float32)
    for kc in range(KC_H):
        for m in range(NSLICE):
            whh_t[:, kc, m, :] = WhhT[kc * 128:(kc + 1) * 128,
                                      m * 128:(m + 1) * 128]
    ident = np.eye(128, dtype=np.float32)

    btot = ((b_ih + b_hh) * gsign).astype(np.float32)
    brz = np.zeros((128, 4), np.float32)
    for m in range(4):
        brz[:, m] = btot[m * 128:(m + 1) * 128]
    biasn = np.zeros((128, 2, B), np.float32)
    for s in range(2):
        biasn[:, s, :] = b_hh[512 + s * 128: 512 + (s + 1) * 128, None]
    biasn = biasn.reshape(128, 2 * B)
    bihn = np.zeros((128, 2), np.float32)
    for s in range(2):
        bihn[:, s] = b_ih[512 + s * 128: 512 + (s + 1) * 128]

    fcw = np.zeros((128, KC_H, N_C), np.float32)
    for kc in range(KC_H):
        fcw[:, kc, :] = fc_w.T[kc * 128:(kc + 1) * 128, :] * HSCALE
    fcb = fc_b.astype(np.float32).reshape(N_C, 1)

    return {
        "xT": xT,
        "wih": wih_t.astype(bf),
        "whh": whh_t.astype(W_NP),
        "ident": ident.astype(bf),
        "biasn": biasn.astype(bf),
        "brz": brz,
        "bihn": bihn,
        "fcw": fcw.astype(bf),
        "fcb": fcb,
    }


def unpack_output(out):
    """out: [N_C, B] -> [B, N_C]"""
    return np.ascontiguousarray(out.T)


# ---------------- harness entry point ----------------
_NC_CACHE = {}

def _get_nc():
    if "nc" not in _NC_CACHE:
        _NC_CACHE["nc"] = build_nc(T=512, TBLK=64, n_cores=8)
    return _NC_CACHE["nc"]


def kernel(x, W_ih, W_hh, b_ih, b_hh, fc_w, fc_b):
    """Full-input GRU classifier on 8 NeuronCores (data-parallel over batch).

    x: [256, 512, 512] fp32 -> returns [256, 101] fp32.
    """
    from concourse.bass_utils import run_bass_kernel_spmd
    x = np.asarray(x, dtype=np.float32)
    W_ih = np.asarray(W_ih, dtype=np.float32)
    W_hh = np.asarray(W_hh, dtype=np.float32)
    b_ih = np.asarray(b_ih, dtype=np.float32)
    b_hh = np.asarray(b_hh, dtype=np.float32)
    fc_w = np.asarray(fc_w, dtype=np.float32)
    fc_b = np.asarray(fc_b, dtype=np.float32)
    nc = _get_nc()
    n_cores = 8
    in_maps = [pack_inputs(x[c * B:(c + 1) * B], W_ih, W_hh, b_ih, b_hh,
                           fc_w, fc_b) for c in range(n_cores)]
    res = run_bass_kernel_spmd(nc, in_maps, core_ids=list(range(n_cores)))
    out = np.concatenate([unpack_output(res.results[c]["out"])
                          for c in range(n_cores)], axis=0)
    return out.astype(np.float32)
